# revision 38
# baseline (speedup 1.0000x reference)
"""Trainium2 Bass kernel for nn_Attention_41729902248209 — v4: all-fp8 DoubleRow.

8-head attention block: x (8, 512, 32, 32) -> QKV proj -> softmax attention
-> out proj + residual. Data-parallel over batch: one batch element per
NeuronCore (8 cores).

All four GEMMs run fp8e4m3 with DoubleRow perf mode (two 128-partition
K-planes per instruction at 0.5 cyc/row = 4x fp16): PE total ~27us vs the
~62us ACT exp floor, so the schedule aims ACT wall-to-wall and squeezes the
head (before the first exp) and tail (after the last exp).

Scaling scheme (all powers of 2, exact in fp):
  - host: x8 = fp8(x); w*8 = fp8(8*W^T); biases 8x in f32 (bqk, bvb).
  - q8/k8/v8 = fp8(psum + 8*bias) = 8*(q+b) -- plain adds, O(8) values.
  - scores psum = 64*logit; pT = fp8(exp(psum*(0.125/64) - 2)); the -2 bias
    keeps exp in e4m3 range (logits O(7)) and cancels in the division.
  - attnv psum: rows 0:64 = 8*num; row 64 = den/2 (the v8 "ones" column is
    0.5) so the reciprocal directly yields rrow = 2/den and
    os8 = fp8(8*num * 2/den) = fp8(16*attn) sits in e4m3's sweet spot.
  - y = psum/128 + xr (fp16 residual with b_last folded), fp16 out.

Numerics: rel err ~1.0e-2 vs the 2e-2 gate (numpy model; CoreSim confirms).

ISA constraints honored: DoubleRow weights APs keep the k-pair dim stride a
multiple of 16 elements (v8 head blocks padded 65->66, s3_lw rule); GPSIMD
never touches PSUM (all psum evacs on DVE/ACT).

Head: one DMA descriptor per tensor (x8 on the gpsimd SWDGE; wq/wv on sync,
wk/wl on scalar HWDGE), projection evacs split DVE (q) / ACT-Identity (k).
Tail: head 7's attnv pairs run inside its own score phase; after the last
exp only two attnv matmuls + denominator chains + the DoubleRow out
projection remain, with its kp=0 half pre-issued and evacs split
DVE / ACT+gpsimd, one output DMA per 128-row tile.
"""

import numpy as np

import concourse.mybir as mybir
import concourse.tile as tile
from concourse import bacc
from concourse.bass_utils import run_bass_kernel_spmd

F8 = mybir.dt.float8e4
F16 = mybir.dt.float16
F32 = mybir.dt.float32

BS = 8
H = 8
D = 64
CIN = 512
N = 1024
NK = CIN // 128  # contraction tiles for cin
NJT = N // 128  # j tiles
NCH = N // 512  # i chunks of 512

AF = mybir.ActivationFunctionType
ALU = mybir.AluOpType
DR = mybir.MatmulPerfMode.DoubleRow


def _emit(tc, d, sb, ps):
    nc = tc.nc

    x8_sb = sb.tile([128, NK, N], F8, tag="x8")
    xr_sb = sb.tile([128, NK, N], F16, tag="xr")
    wq_sb = sb.tile([128, NK, 512], F8, tag="wq")
    wk_sb = sb.tile([128, NK, 512], F8, tag="wk")
    wvl_sb = sb.tile([128, NK, 1024], F8, tag="wvl")
    bb_sb = sb.tile([128, 520], F32, tag="bb")
    eye_sb = sb.tile([128, 128], F16, tag="eye")
    q8_sb = sb.tile([128, 2, 2 * N], F8, tag="q8")  # (pl, g*N + i)
    k8_sb = sb.tile([128, 2, 2 * N], F8, tag="k8")
    v8_sb = sb.tile([128, NJT, H, D + 2], F8, tag="v8")
    # two tiles (head pairs 0,1 | 2,3) so the out-projection kp=0 half is
    # not dep-serialized behind the last heads' os writes
    os_lo = sb.tile([128, 2, N], F8, tag="os_lo")
    os_hi = sb.tile([128, 2, N], F8, tag="os_hi")
    ebias_sb = sb.tile([128, 1], F32, tag="ebias")
    nc.gpsimd.memset(ebias_sb[:], -2.0)

    # --- input DMAs: one descriptor per tensor, spread across queues ---
    def kfold(ap):  # dram (128k+p, i) -> sbuf (p, k, i)
        return ap.rearrange("(k p) i -> p k i", p=128)

    nc.gpsimd.dma_start(x8_sb[:], kfold(d["x8"].ap()))
    nc.sync.dma_start(wq_sb[:], kfold(d["wq"].ap()))
    nc.scalar.dma_start(wk_sb[:], kfold(d["wk"].ap()))
    nc.sync.dma_start(bb_sb[:], d["bb"].ap())
    nc.sync.dma_start(wvl_sb[:], kfold(d["wvl"].ap()))
    nc.gpsimd.dma_start(xr_sb[:], kfold(d["xr"].ap()))
    nc.sync.dma_start(eye_sb[:], d["eye"].ap())
    # 0.5 columns for v~ (column 64 of each head block): row 64 of the attnv
    # psum integrates den/2, so its reciprocal is directly 2/den.
    nc.vector.memset(v8_sb[:, :, :, D : D + 1], 0.5)

    # --- stage emitters ---
    def qk_quarter(wsb, dst, bcol, t, c, eng="v"):
        """One 128-feature x 512-token chunk of the q/k projection + bias,
        evacuated to fp8 on DVE (eng='v') or ACT Identity (eng='s')."""
        g, pl = divmod(t, 2)
        p = ps.tile([128, 512], F32, tag="mm", name=f"qk{t}_{bcol}_{c}")
        for kp in range(2):
            nc.tensor.matmul(
                p[:],
                wsb[:, 2 * kp : 2 * kp + 2, 128 * t : 128 * t + 128],
                x8_sb[:, 2 * kp : 2 * kp + 2, 512 * c : 512 * c + 512],
                start=(kp == 0),
                stop=(kp == 1),
                perf_mode=DR,
            )
        dsl = dst[:, pl, g * N + 512 * c : g * N + 512 * c + 512]
        if eng == "v":
            nc.vector.tensor_scalar_add(dsl, p[:], bb_sb[:, bcol : bcol + 1])
        else:
            nc.scalar.activation(
                dsl, p[:], AF.Identity, bias=bb_sb[:, bcol : bcol + 1]
            )

    def v_tile(jt):
        """Project v for token tile jt: (128 tokens, 512 feats) + 8*b_v -> fp8."""
        p = ps.tile([128, 512], F32, tag="mm")
        for kp in range(2):
            nc.tensor.matmul(
                p[:],
                x8_sb[:, 2 * kp : 2 * kp + 2, 128 * jt : 128 * jt + 128],
                wvl_sb[:, 2 * kp : 2 * kp + 2, 0:512],
                start=(kp == 0),
                stop=(kp == 1),
                perf_mode=DR,
            )
        nc.vector.tensor_tensor(
            v8_sb[:, jt, :, 0:D],
            p[:].rearrange("p (h e) -> p h e", e=D),
            bb_sb[:, 8:520].rearrange("p (h e) -> p h e", e=D),
            ALU.add,
        )

    pt_tiles = {}

    def scores_exp(h, jts):
        """scoresT (j, i) for head h via DoubleRow fp8 + exp -> pT fp8."""
        g, a = divmod(h, 4)
        m0 = 32 * a
        if h in pt_tiles:
            pT = pt_tiles[h]
        else:
            pT = sbuf_pt_pool.tile([128, NJT, N], F8, tag="pt", name=f"pt{h}")
            pt_tiles[h] = pT
        for jt in jts:
            sp = ps.tile([128, N], F32, tag="score", bufs=2)
            for c in range(NCH):
                nc.tensor.matmul(
                    sp[:, 512 * c : 512 * c + 512],
                    k8_sb[m0 : m0 + 32, :, g * N + 128 * jt : g * N + 128 * jt + 128],
                    q8_sb[m0 : m0 + 32, :, g * N + 512 * c : g * N + 512 * c + 512],
                    start=True,
                    stop=True,
                    perf_mode=DR,
                    tile_position=(m0, 0),
                )
            nc.scalar.activation(
                pT[:, jt, :], sp[:], AF.Exp, bias=ebias_sb[:], scale=0.125 / 64.0
            )

    pv_tiles = {}

    def attnv_unit(h, c, g2):
        """One DoubleRow matmul (j-tiles 2*g2, 2*g2+1) of outT~ for (h, c);
        evac + denominator extraction after the last unit of the chunk."""
        pr, hh = divmod(h, 2)
        pT = pt_tiles[h]
        key = (h, c)
        if key not in pv_tiles:
            pv_tiles[key] = ps.tile([128, 512], F32, tag="mm", name=f"av{h}_{c}")
        p = pv_tiles[key]
        nc.tensor.matmul(
            p[0:65, :],
            v8_sb[:, 2 * g2 : 2 * g2 + 2, h, 0 : D + 1],
            pT[:, 2 * g2 : 2 * g2 + 2, 512 * c : 512 * c + 512],
            start=(g2 == 0),
            stop=(g2 == 3),
            perf_mode=DR,
        )
        if g2 == 3:
            del pv_tiles[key]
            if c == NCH - 1:
                del pt_tiles[h]
            r = 2 * h + c
            # denominator chain, DMA-free: recip psum row (p64) = 2/den ->
            # sbuf f32 (p64), 1-partition cross-quadrant copy p64 -> p0,
            # gpsimd broadcast, one DVE multiply straight off PSUM into os8.
            rrow = rr_pool.tile([1, 512], F32, tag="rrow", name=f"rr{r}")
            nc.vector.reciprocal(rrow[0:1, :], p[64:65, :])
            rb = rb_pool.tile([128, 512], F32, tag="rb", name=f"rb{r}")
            nc.gpsimd.partition_broadcast(rb[:], rrow[0:1, :])
            ot = os_lo if pr < 2 else os_hi
            nc.vector.tensor_tensor(
                ot[64 * hh : 64 * hh + 64, pr % 2, 512 * c : 512 * c + 512],
                p[0:64, :],
                rb[0:64, :],
                ALU.mult,
            )

    def op_psums():
        """Out-projection psum chunks: ct0/ct1 as bank-halves of score-tag
        tiles, ct2/ct3 as mm-tag tiles (all freed by the tail)."""
        chunks = {}
        fulls = {}
        for ct in range(2):
            p = ps.tile([128, N], F32, tag="score", bufs=2, name=f"yp{ct}")
            fulls[ct] = p
            for c in range(NCH):
                chunks[(ct, c)] = p[:, 512 * c : 512 * c + 512]
        for ct in range(2, 4):
            for c in range(NCH):
                p = ps.tile([128, 512], F32, tag="mm", name=f"yp{ct}_{c}")
                chunks[(ct, c)] = p[:]
        return chunks, fulls

    def op_xr(chunks, ct, c):
        """psum = 128*xr via an fp16 identity matmul on the idle tail PE, so
        each evac is a single scale op with no separate residual add."""
        nc.tensor.matmul(
            chunks[(ct, c)],
            eye_sb[:],
            xr_sb[:, ct, 512 * c : 512 * c + 512],
            start=True,
            stop=False,
        )

    def op_mm(chunks, ct, c, kp):
        ot = os_lo if kp == 0 else os_hi
        nc.tensor.matmul(
            chunks[(ct, c)],
            wvl_sb[:, 2 * kp : 2 * kp + 2, 512 + 128 * ct : 512 + 128 * ct + 128],
            ot[:, :, 512 * c : 512 * c + 512],
            start=False,
            stop=(kp == 1),
            perf_mode=DR,
        )

    # --- pools that emitters close over ---
    import contextlib

    stack = contextlib.ExitStack()
    sbuf_pt_pool = stack.enter_context(tc.tile_pool(name="pt", bufs=3))
    rb_pool = stack.enter_context(tc.tile_pool(name="rb", bufs=3))
    rr_pool = stack.enter_context(tc.tile_pool(name="rr", bufs=3))
    y_pool = stack.enter_context(tc.tile_pool(name="y", bufs=4))

    # --- software-pipelined emission (keep ACT continuously fed) ---
    av_units = [
        (h, c, g2) for h in range(H - 1) for c in range(NCH) for g2 in range(4)
    ]
    av_pos = 0

    def drain_av(n):
        nonlocal av_pos
        for _ in range(n):
            if av_pos >= len(av_units):
                return
            h, c, g2 = av_units[av_pos]
            av_pos += 1
            attnv_unit(h, c, g2)

    # head: tiles 0,1 of q and k, c0 first, q evacs on DVE / k evacs on ACT
    for c in range(NCH):
        for t in (0, 1):
            qk_quarter(wq_sb, q8_sb, t, t, c, eng="v")
            qk_quarter(wk_sb, k8_sb, 4 + t, t, c, eng="s")
    qk_quarters = [
        (wsb, dst, bcol, t, c)
        for t in (2, 3)
        for (wsb, dst, bcol) in ((wq_sb, q8_sb, t), (wk_sb, k8_sb, 4 + t))
        for c in range(NCH)
    ]
    for g in range(NJT):  # head 0 scores + qk tiles 2,3 (one quarter per step)
        scores_exp(0, [g])
        qk_quarter(*qk_quarters[g])
    for g in range(NJT):  # head 1 scores + v tiles
        scores_exp(1, [g])
        v_tile(g)
    rates = {2: 8, 3: 9, 4: 10, 5: 10, 6: 11, 7: 8}
    for h in range(2, H):
        # front-load each phase's drains (2 per score step) so the last
        # denominator chains overlap the remaining exps instead of the tail
        per = [min(2, max(0, rates[h] - 2 * g)) for g in range(NJT)]
        for g in range(NJT):
            scores_exp(h, [g])
            drain_av(per[g])
            if h == H - 1 and g in (1, 3, 5):
                # head 7's early attnv pairs, right after their exps land
                g2 = (g - 1) // 2
                attnv_unit(7, 0, g2)
                attnv_unit(7, 1, g2)
    drain_av(len(av_units))  # any remainder of heads 0-6
    attnv_unit(7, 0, 3)  # final pair + denominators
    attnv_unit(7, 1, 3)

    # tail: out projection. kp=0 (head pairs 0,1) only needs early os tiles;
    # kp=1 c-chunks chase the head-7 denominator chains.
    chunks, fulls = op_psums()
    # ct0/ct1 (score-tag psums, free right after the last exp) run xr+kp0 at
    # once; their kp1-c0 follows as soon as the (7,0) os write lands — ahead
    # of ct2/ct3 whose mm-tag slots wait on the head-7 pv releases anyway.
    for ct in range(2):
        for c in range(NCH):
            op_xr(chunks, ct, c)
            op_mm(chunks, ct, c, 0)
    op_mm(chunks, 0, 0, 1)
    op_mm(chunks, 1, 0, 1)
    for ct in range(2, 4):
        for c in range(NCH):
            op_xr(chunks, ct, c)
            op_mm(chunks, ct, c, 0)
    op_mm(chunks, 2, 0, 1)
    op_mm(chunks, 3, 0, 1)
    for ct in range(4):
        op_mm(chunks, ct, 1, 1)
    # evac: residual already in psum, so one scale op per region; 512-wide
    # ct2/ct3 halves go first (ready earliest), 1024-wide ct0/ct1 follow;
    # 4 per-ct DMAs split over SP/ACT queues (ACT is idle by then).
    y_tiles = {
        ct: y_pool.tile([128, N], F16, tag="y", name=f"y{ct}") for ct in range(4)
    }
    SCL = 1.0 / 128.0

    def ydma(ct, q):
        q.dma_start(d["y"].ap()[128 * ct : 128 * ct + 128, :], y_tiles[ct][:])

    nc.scalar.activation(
        y_tiles[2][:, 0:512], chunks[(2, 0)], AF.Identity, scale=SCL
    )
    nc.vector.tensor_scalar_mul(y_tiles[3][:, 0:512], chunks[(3, 0)], SCL)
    nc.scalar.activation(
        y_tiles[2][:, 512:1024], chunks[(2, 1)], AF.Identity, scale=SCL
    )
    ydma(2, nc.sync)
    nc.vector.tensor_scalar_mul(y_tiles[3][:, 512:1024], chunks[(3, 1)], SCL)
    ydma(3, nc.scalar)
    nc.vector.tensor_scalar_mul(y_tiles[0][:], fulls[0][:], SCL)
    ydma(0, nc.sync)
    nc.scalar.activation(y_tiles[1][:], fulls[1][:], AF.Identity, scale=SCL)
    ydma(1, nc.scalar)

    stack.close()


def _build(loop=1):
    nc = bacc.Bacc("TRN2", target_bir_lowering=False, debug=False, num_devices=BS)
    d = {}
    d["x8"] = nc.dram_tensor("x8", [CIN, N], F8, kind="ExternalInput")
    d["xr"] = nc.dram_tensor("xr", [CIN, N], F16, kind="ExternalInput")
    d["wq"] = nc.dram_tensor("wq", [CIN, 512], F8, kind="ExternalInput")
    d["wk"] = nc.dram_tensor("wk", [CIN, 512], F8, kind="ExternalInput")
    d["wvl"] = nc.dram_tensor("wvl", [CIN, 1024], F8, kind="ExternalInput")
    d["bb"] = nc.dram_tensor("bb", [128, 520], F32, kind="ExternalInput")
    d["eye"] = nc.dram_tensor("eye", [128, 128], F16, kind="ExternalInput")
    d["y"] = nc.dram_tensor("y", [CIN, N], F16, kind="ExternalOutput")

    with tile.TileContext(nc) as tc:
        with (
            tc.tile_pool(name="sb", bufs=1) as sb,
            tc.tile_pool(name="ps", bufs=4, space="PSUM") as ps,
        ):
            for i in range(loop):
                if i:
                    with tc.tile_critical():
                        nc.all_engine_barrier()
                _emit(tc, d, sb, ps)
    nc.compile()
    return nc


_NC_CACHE = {}


def get_nc(loop=1):
    if loop not in _NC_CACHE:
        _NC_CACHE[loop] = _build(loop)
    return _NC_CACHE[loop]


def host_prep(x, W_fc, b_fc, W_last, b_last):
    """Full inputs -> list of 8 per-core input maps."""
    import ml_dtypes

    f8 = ml_dtypes.float8_e4m3fn
    x = np.asarray(x, dtype=np.float32)
    W_fc = np.asarray(W_fc, dtype=np.float32)
    b_fc = np.asarray(b_fc, dtype=np.float32)
    W_last = np.asarray(W_last, dtype=np.float32)
    b_last = np.asarray(b_last, dtype=np.float32)

    hh = np.arange(H).repeat(D) * 3 * D  # 192h per f'=64h+d
    dd = np.tile(np.arange(D), H)
    pq, pk, pv = hh + dd, hh + D + dd, hh + 2 * D + dd

    # plane-major column permutation for q/k: new col t*128+m <- feature
    # f = 64h + d with h = 4*(t//2) + m//32, d = 32*(t%2) + m%32
    t = np.arange(512) // 128
    m = np.arange(512) % 128
    perm = 64 * (4 * (t // 2) + m // 32) + 32 * (t % 2) + m % 32

    wq = np.ascontiguousarray((8 * W_fc[pq][perm]).T).astype(f8)
    wk = np.ascontiguousarray((8 * W_fc[pk][perm]).T).astype(f8)
    wv = np.ascontiguousarray((8 * W_fc[pv]).T).astype(f8)
    wl = np.ascontiguousarray((8 * W_last.T)).astype(f8)
    wvl = np.ascontiguousarray(np.concatenate([wv, wl], axis=1))
    bq, bk, bv = 8 * b_fc[pq][perm], 8 * b_fc[pk][perm], 8 * b_fc[pv]
    bqk = np.ascontiguousarray(
        np.concatenate([bq.reshape(4, 128).T, bk.reshape(4, 128).T], axis=1)
    ).astype(np.float32)
    bvb = np.ascontiguousarray(np.tile(bv[None, :], (128, 1))).astype(np.float32)
    bb = np.ascontiguousarray(np.concatenate([bqk, bvb], axis=1)).astype(np.float32)
    eye = (128.0 * np.eye(128)).astype(np.float16)

    xf = x.reshape(BS, CIN, N)
    maps = []
    for b in range(BS):
        maps.append(
            {
                "x8": xf[b].astype(f8),
                "xr": (xf[b] + b_last[:, None]).astype(np.float16),
                "wq": wq,
                "wk": wk,
                "wvl": wvl,
                "bb": bb,
                "eye": eye,
            }
        )
    return maps


def kernel(x, W_fc, b_fc, W_last, b_last):
    nc = get_nc()
    maps = host_prep(x, W_fc, b_fc, W_last, b_last)
    res = run_bass_kernel_spmd(nc, maps, core_ids=list(range(BS)))
    y = np.stack([res.results[b]["y"] for b in range(BS)]).astype(np.float32)
    return y.reshape(BS, CIN, 32, 32)



# revision 40
# speedup vs baseline: 1.0239x; 1.0239x over previous
"""Trainium2 Bass kernel for nn_Attention_41729902248209 — v4: all-fp8 DoubleRow.

8-head attention block: x (8, 512, 32, 32) -> QKV proj -> softmax attention
-> out proj + residual. Data-parallel over batch: one batch element per
NeuronCore (8 cores).

All four GEMMs run fp8e4m3 with DoubleRow perf mode (two 128-partition
K-planes per instruction at 0.5 cyc/row = 4x fp16): PE total ~27us vs the
~62us ACT exp floor, so the schedule aims ACT wall-to-wall and squeezes the
head (before the first exp) and tail (after the last exp).

Scaling scheme (all powers of 2, exact in fp):
  - host: x8 = fp8(x); w*8 = fp8(8*W^T); biases 8x in f32 (bqk, bvb).
  - q8/k8/v8 = fp8(psum + 8*bias) = 8*(q+b) -- plain adds, O(8) values.
  - scores psum = 64*logit; pT = fp8(exp(psum*(0.125/64) - 2)); the -2 bias
    keeps exp in e4m3 range (logits O(7)) and cancels in the division.
  - attnv psum: rows 0:64 = 8*num; row 64 = den/2 (the v8 "ones" column is
    0.5) so the reciprocal directly yields rrow = 2/den and
    os8 = fp8(8*num * 2/den) = fp8(16*attn) sits in e4m3's sweet spot.
  - y = psum/128 + xr (fp16 residual with b_last folded), fp16 out.

Numerics: rel err ~1.0e-2 vs the 2e-2 gate (numpy model; CoreSim confirms).

ISA constraints honored: DoubleRow weights APs keep the k-pair dim stride a
multiple of 16 elements (v8 head blocks padded 65->66, s3_lw rule); GPSIMD
never touches PSUM (all psum evacs on DVE/ACT).

Head: one DMA descriptor per tensor (x8 on the gpsimd SWDGE; wq/wv on sync,
wk/wl on scalar HWDGE), projection evacs split DVE (q) / ACT-Identity (k).
Tail: head 7's attnv pairs run inside its own score phase; after the last
exp only two attnv matmuls + denominator chains + the DoubleRow out
projection remain, with its kp=0 half pre-issued and evacs split
DVE / ACT+gpsimd, one output DMA per 128-row tile.
"""

import numpy as np

import concourse.mybir as mybir
import concourse.tile as tile
from concourse import bacc
from concourse.bass_utils import run_bass_kernel_spmd

F8 = mybir.dt.float8e4
F16 = mybir.dt.float16
F32 = mybir.dt.float32

BS = 8
H = 8
D = 64
CIN = 512
N = 1024
NK = CIN // 128  # contraction tiles for cin
NJT = N // 128  # j tiles
NCH = N // 512  # i chunks of 512

AF = mybir.ActivationFunctionType
ALU = mybir.AluOpType
DR = mybir.MatmulPerfMode.DoubleRow


def _emit(tc, d, sb, ps):
    nc = tc.nc

    x8_sb = sb.tile([128, NK, N], F8, tag="x8")
    xr_sb = sb.tile([128, NK, N], F16, tag="xr")
    wq_sb = sb.tile([128, NK, 512], F8, tag="wq")
    wk_sb = sb.tile([128, NK, 512], F8, tag="wk")
    wvl_sb = sb.tile([128, NK, 1024], F8, tag="wvl")
    bb_sb = sb.tile([128, 520], F32, tag="bb")
    q8_sb = sb.tile([128, 2, 2 * N], F8, tag="q8")  # (pl, g*N + i)
    k8_sb = sb.tile([128, 2, 2 * N], F8, tag="k8")
    v8_sb = sb.tile([128, NJT, H, D + 2], F8, tag="v8")
    # two tiles (head pairs 0,1 | 2,3) so the out-projection kp=0 half is
    # not dep-serialized behind the last heads' os writes
    os_lo = sb.tile([128, 2, N], F8, tag="os_lo")
    os_hi = sb.tile([128, 2, N], F8, tag="os_hi")
    ebias_sb = sb.tile([128, 1], F32, tag="ebias")
    nc.gpsimd.memset(ebias_sb[:], -2.0)

    # --- input DMAs: one descriptor per tensor, spread across queues ---
    def kfold(ap):  # dram (128k+p, i) -> sbuf (p, k, i)
        return ap.rearrange("(k p) i -> p k i", p=128)

    nc.gpsimd.dma_start(x8_sb[:], kfold(d["x8"].ap()))
    nc.sync.dma_start(wq_sb[:], kfold(d["wq"].ap()))
    nc.scalar.dma_start(wk_sb[:], kfold(d["wk"].ap()))
    nc.sync.dma_start(bb_sb[:], d["bb"].ap())
    nc.sync.dma_start(wvl_sb[:], kfold(d["wvl"].ap()))
    nc.gpsimd.dma_start(xr_sb[:], kfold(d["xr"].ap()))
    # 0.5 columns for v~ (column 64 of each head block): row 64 of the attnv
    # psum integrates den/2, so its reciprocal is directly 2/den.
    nc.vector.memset(v8_sb[:, :, :, D : D + 1], 0.5)

    # --- stage emitters ---
    def qk_quarter(wsb, dst, bcol, t, c, eng="v"):
        """One 128-feature x 512-token chunk of the q/k projection + bias,
        evacuated to fp8 on DVE (eng='v') or ACT Identity (eng='s')."""
        g, pl = divmod(t, 2)
        p = ps.tile([128, 512], F32, tag="mm", name=f"qk{t}_{bcol}_{c}")
        for kp in range(2):
            nc.tensor.matmul(
                p[:],
                wsb[:, 2 * kp : 2 * kp + 2, 128 * t : 128 * t + 128],
                x8_sb[:, 2 * kp : 2 * kp + 2, 512 * c : 512 * c + 512],
                start=(kp == 0),
                stop=(kp == 1),
                perf_mode=DR,
            )
        dsl = dst[:, pl, g * N + 512 * c : g * N + 512 * c + 512]
        if eng == "v":
            nc.vector.tensor_scalar_add(dsl, p[:], bb_sb[:, bcol : bcol + 1])
        else:
            nc.scalar.activation(
                dsl, p[:], AF.Identity, bias=bb_sb[:, bcol : bcol + 1]
            )

    def v_tile(jt):
        """Project v for token tile jt: (128 tokens, 512 feats) + 8*b_v -> fp8."""
        p = ps.tile([128, 512], F32, tag="mm")
        for kp in range(2):
            nc.tensor.matmul(
                p[:],
                x8_sb[:, 2 * kp : 2 * kp + 2, 128 * jt : 128 * jt + 128],
                wvl_sb[:, 2 * kp : 2 * kp + 2, 0:512],
                start=(kp == 0),
                stop=(kp == 1),
                perf_mode=DR,
            )
        nc.vector.tensor_tensor(
            v8_sb[:, jt, :, 0:D],
            p[:].rearrange("p (h e) -> p h e", e=D),
            bb_sb[:, 8:520].rearrange("p (h e) -> p h e", e=D),
            ALU.add,
        )

    pt_tiles = {}

    def scores_exp(h, jts):
        """scoresT (j, i) for head h via DoubleRow fp8 + exp -> pT fp8."""
        g, a = divmod(h, 4)
        m0 = 32 * a
        if h in pt_tiles:
            pT = pt_tiles[h]
        else:
            pT = sbuf_pt_pool.tile([128, NJT, N], F8, tag="pt", name=f"pt{h}")
            pt_tiles[h] = pT
        for jt in jts:
            sp = ps.tile([128, N], F32, tag="score", bufs=2)
            for c in range(NCH):
                nc.tensor.matmul(
                    sp[:, 512 * c : 512 * c + 512],
                    k8_sb[m0 : m0 + 32, :, g * N + 128 * jt : g * N + 128 * jt + 128],
                    q8_sb[m0 : m0 + 32, :, g * N + 512 * c : g * N + 512 * c + 512],
                    start=True,
                    stop=True,
                    perf_mode=DR,
                    tile_position=(m0, 0),
                )
            nc.scalar.activation(
                pT[:, jt, :], sp[:], AF.Exp, bias=ebias_sb[:], scale=0.125 / 64.0
            )

    pv_tiles = {}

    def attnv_unit(h, c, g2):
        """One DoubleRow matmul (j-tiles 2*g2, 2*g2+1) of outT~ for (h, c);
        evac + denominator extraction after the last unit of the chunk."""
        pr, hh = divmod(h, 2)
        pT = pt_tiles[h]
        key = (h, c)
        if key not in pv_tiles:
            pv_tiles[key] = ps.tile([128, 512], F32, tag="mm", name=f"av{h}_{c}")
        p = pv_tiles[key]
        nc.tensor.matmul(
            p[0:65, :],
            v8_sb[:, 2 * g2 : 2 * g2 + 2, h, 0 : D + 1],
            pT[:, 2 * g2 : 2 * g2 + 2, 512 * c : 512 * c + 512],
            start=(g2 == 0),
            stop=(g2 == 3),
            perf_mode=DR,
        )
        if g2 == 3:
            del pv_tiles[key]
            if c == NCH - 1:
                del pt_tiles[h]
            r = 2 * h + c
            # denominator chain, DMA-free: recip psum row (p64) = 2/den ->
            # sbuf f32 (p64), 1-partition cross-quadrant copy p64 -> p0,
            # gpsimd broadcast, one DVE multiply straight off PSUM into os8.
            rrow = rr_pool.tile([1, 512], F32, tag="rrow", name=f"rr{r}")
            nc.vector.reciprocal(rrow[0:1, :], p[64:65, :])
            rb = rb_pool.tile([128, 512], F32, tag="rb", name=f"rb{r}")
            nc.gpsimd.partition_broadcast(rb[:], rrow[0:1, :])
            ot = os_lo if pr < 2 else os_hi
            nc.vector.tensor_tensor(
                ot[64 * hh : 64 * hh + 64, pr % 2, 512 * c : 512 * c + 512],
                p[0:64, :],
                rb[0:64, :],
                ALU.mult,
            )

    def op_psums():
        """Out-projection psum chunks: ct0/ct1 as bank-halves of score-tag
        tiles, ct2/ct3 as mm-tag tiles (all freed by the tail)."""
        chunks = {}
        fulls = {}
        for ct in range(2):
            p = ps.tile([128, N], F32, tag="score", bufs=2, name=f"yp{ct}")
            fulls[ct] = p
            for c in range(NCH):
                chunks[(ct, c)] = p[:, 512 * c : 512 * c + 512]
        for ct in range(2, 4):
            for c in range(NCH):
                p = ps.tile([128, 512], F32, tag="mm", name=f"yp{ct}_{c}")
                chunks[(ct, c)] = p[:]
        return chunks, fulls

    def op_mm(chunks, ct, c, kp):
        ot = os_lo if kp == 0 else os_hi
        nc.tensor.matmul(
            chunks[(ct, c)],
            wvl_sb[:, 2 * kp : 2 * kp + 2, 512 + 128 * ct : 512 + 128 * ct + 128],
            ot[:, :, 512 * c : 512 * c + 512],
            start=(kp == 0),
            stop=(kp == 1),
            perf_mode=DR,
        )

    # --- pools that emitters close over ---
    import contextlib

    stack = contextlib.ExitStack()
    sbuf_pt_pool = stack.enter_context(tc.tile_pool(name="pt", bufs=3))
    rb_pool = stack.enter_context(tc.tile_pool(name="rb", bufs=3))
    rr_pool = stack.enter_context(tc.tile_pool(name="rr", bufs=3))
    y_pool = stack.enter_context(tc.tile_pool(name="y", bufs=4))
    yh_pool = stack.enter_context(tc.tile_pool(name="yh", bufs=4))

    # --- software-pipelined emission (keep ACT continuously fed) ---
    av_units = [
        (h, c, g2) for h in range(H - 1) for c in range(NCH) for g2 in range(4)
    ]
    av_pos = 0

    def drain_av(n):
        nonlocal av_pos
        for _ in range(n):
            if av_pos >= len(av_units):
                return
            h, c, g2 = av_units[av_pos]
            av_pos += 1
            attnv_unit(h, c, g2)

    # head: tiles 0,1 of q and k, c0 first, q evacs on DVE / k evacs on ACT
    for c in range(NCH):
        for t in (0, 1):
            qk_quarter(wq_sb, q8_sb, t, t, c, eng="v")
            qk_quarter(wk_sb, k8_sb, 4 + t, t, c, eng="s")
    qk_quarters = [
        (wsb, dst, bcol, t, c)
        for t in (2, 3)
        for (wsb, dst, bcol) in ((wq_sb, q8_sb, t), (wk_sb, k8_sb, 4 + t))
        for c in range(NCH)
    ]
    for g in range(NJT):  # head 0 scores + qk tiles 2,3 (one quarter per step)
        scores_exp(0, [g])
        qk_quarter(*qk_quarters[g])
    for g in range(NJT):  # head 1 scores + v tiles
        scores_exp(1, [g])
        v_tile(g)
    rates = {2: 8, 3: 9, 4: 10, 5: 10, 6: 11, 7: 8}
    for h in range(2, H):
        # front-load each phase's drains (2 per score step) so the last
        # denominator chains overlap the remaining exps instead of the tail
        per = [min(2, max(0, rates[h] - 2 * g)) for g in range(NJT)]
        for g in range(NJT):
            scores_exp(h, [g])
            drain_av(per[g])
            if h == H - 1 and g in (1, 3, 5):
                # head 7's early attnv pairs, right after their exps land
                g2 = (g - 1) // 2
                attnv_unit(7, 0, g2)
                attnv_unit(7, 1, g2)
    drain_av(len(av_units))  # any remainder of heads 0-6
    attnv_unit(7, 0, 3)  # final pair + denominators
    attnv_unit(7, 1, 3)

    # tail: out projection. kp=0 (head pairs 0,1) only needs early os tiles;
    # kp=1 c-chunks chase the head-7 denominator chains.
    chunks, fulls = op_psums()
    # ct0/ct1 (score-tag psums, free right after the last exp) run kp0 at
    # once; their kp1-c0 follows as soon as the (7,0) os write lands — ahead
    # of ct2/ct3 whose mm-tag slots wait on the head-7 pv releases anyway.
    for ct in range(2):
        for c in range(NCH):
            op_mm(chunks, ct, c, 0)
    op_mm(chunks, 0, 0, 1)
    op_mm(chunks, 1, 0, 1)
    for ct in range(2, 4):
        for c in range(NCH):
            op_mm(chunks, ct, c, 0)
    op_mm(chunks, 2, 0, 1)
    op_mm(chunks, 3, 0, 1)
    for ct in range(4):
        op_mm(chunks, ct, 1, 1)
    # evac c-major (c0 chunks overlap the (7,1) denominator chain; alternating
    # DVE-direct / ACT+DVE paths); each ct's output DMA launches right after
    # its c1 half lands
    y_tiles = {
        ct: y_pool.tile([128, N], F16, tag="y", name=f"y{ct}") for ct in range(4)
    }
    SCL = 1.0 / 128.0
    # ct2/ct3 512-chunks evac first (ready one den-chain earlier), then the
    # 1024-wide ct0/ct1 ops (half the instruction count of c-split evacs);
    # DMAs per ct on SP + (tail-idle) ACT queues.
    for c in range(NCH):
        t = yh_pool.tile([128, 512], F16, tag="yh", name=f"yh2_{c}")
        nc.scalar.activation(t[:], chunks[(2, c)], AF.Identity, scale=SCL)
        nc.vector.tensor_tensor(
            y_tiles[2][:, 512 * c : 512 * c + 512], t[:],
            xr_sb[:, 2, 512 * c : 512 * c + 512], ALU.add,
        )
        nc.vector.scalar_tensor_tensor(
            y_tiles[3][:, 512 * c : 512 * c + 512], chunks[(3, c)], SCL,
            xr_sb[:, 3, 512 * c : 512 * c + 512], ALU.mult, ALU.add,
        )
    nc.sync.dma_start(d["y"].ap()[256:384, :], y_tiles[2][:])
    nc.scalar.dma_start(d["y"].ap()[384:512, :], y_tiles[3][:])
    nc.vector.scalar_tensor_tensor(
        y_tiles[0][:], fulls[0][:], SCL, xr_sb[:, 0, :], ALU.mult, ALU.add
    )
    nc.sync.dma_start(d["y"].ap()[0:128, :], y_tiles[0][:])
    th = yh_pool.tile([128, N], F16, tag="yh1", name="yh1")
    nc.scalar.activation(th[:], fulls[1][:], AF.Identity, scale=SCL)
    nc.vector.tensor_tensor(y_tiles[1][:], th[:], xr_sb[:, 1, :], ALU.add)
    nc.scalar.dma_start(d["y"].ap()[128:256, :], y_tiles[1][:])

    stack.close()


def _build(loop=1):
    nc = bacc.Bacc("TRN2", target_bir_lowering=False, debug=False, num_devices=BS)
    d = {}
    d["x8"] = nc.dram_tensor("x8", [CIN, N], F8, kind="ExternalInput")
    d["xr"] = nc.dram_tensor("xr", [CIN, N], F16, kind="ExternalInput")
    d["wq"] = nc.dram_tensor("wq", [CIN, 512], F8, kind="ExternalInput")
    d["wk"] = nc.dram_tensor("wk", [CIN, 512], F8, kind="ExternalInput")
    d["wvl"] = nc.dram_tensor("wvl", [CIN, 1024], F8, kind="ExternalInput")
    d["bb"] = nc.dram_tensor("bb", [128, 520], F32, kind="ExternalInput")
    d["y"] = nc.dram_tensor("y", [CIN, N], F16, kind="ExternalOutput")

    with tile.TileContext(nc) as tc:
        with (
            tc.tile_pool(name="sb", bufs=1) as sb,
            tc.tile_pool(name="ps", bufs=4, space="PSUM") as ps,
        ):
            for i in range(loop):
                if i:
                    with tc.tile_critical():
                        nc.all_engine_barrier()
                _emit(tc, d, sb, ps)
    nc.compile()
    return nc


_NC_CACHE = {}


def get_nc(loop=1):
    if loop not in _NC_CACHE:
        _NC_CACHE[loop] = _build(loop)
    return _NC_CACHE[loop]


def host_prep(x, W_fc, b_fc, W_last, b_last):
    """Full inputs -> list of 8 per-core input maps."""
    import ml_dtypes

    f8 = ml_dtypes.float8_e4m3fn
    x = np.asarray(x, dtype=np.float32)
    W_fc = np.asarray(W_fc, dtype=np.float32)
    b_fc = np.asarray(b_fc, dtype=np.float32)
    W_last = np.asarray(W_last, dtype=np.float32)
    b_last = np.asarray(b_last, dtype=np.float32)

    hh = np.arange(H).repeat(D) * 3 * D  # 192h per f'=64h+d
    dd = np.tile(np.arange(D), H)
    pq, pk, pv = hh + dd, hh + D + dd, hh + 2 * D + dd

    # plane-major column permutation for q/k: new col t*128+m <- feature
    # f = 64h + d with h = 4*(t//2) + m//32, d = 32*(t%2) + m%32
    t = np.arange(512) // 128
    m = np.arange(512) % 128
    perm = 64 * (4 * (t // 2) + m // 32) + 32 * (t % 2) + m % 32

    wq = np.ascontiguousarray((8 * W_fc[pq][perm]).T).astype(f8)
    wk = np.ascontiguousarray((8 * W_fc[pk][perm]).T).astype(f8)
    wv = np.ascontiguousarray((8 * W_fc[pv]).T).astype(f8)
    wl = np.ascontiguousarray((8 * W_last.T)).astype(f8)
    wvl = np.ascontiguousarray(np.concatenate([wv, wl], axis=1))
    bq, bk, bv = 8 * b_fc[pq][perm], 8 * b_fc[pk][perm], 8 * b_fc[pv]
    bqk = np.ascontiguousarray(
        np.concatenate([bq.reshape(4, 128).T, bk.reshape(4, 128).T], axis=1)
    ).astype(np.float32)
    bvb = np.ascontiguousarray(np.tile(bv[None, :], (128, 1))).astype(np.float32)
    bb = np.ascontiguousarray(np.concatenate([bqk, bvb], axis=1)).astype(np.float32)

    xf = x.reshape(BS, CIN, N)
    maps = []
    for b in range(BS):
        maps.append(
            {
                "x8": xf[b].astype(f8),
                "xr": (xf[b] + b_last[:, None]).astype(np.float16),
                "wq": wq,
                "wk": wk,
                "wvl": wvl,
                "bb": bb,
            }
        )
    return maps


def kernel(x, W_fc, b_fc, W_last, b_last):
    nc = get_nc()
    maps = host_prep(x, W_fc, b_fc, W_last, b_last)
    res = run_bass_kernel_spmd(nc, maps, core_ids=list(range(BS)))
    y = np.stack([res.results[b]["y"] for b in range(BS)]).astype(np.float32)
    return y.reshape(BS, CIN, 32, 32)

